# revision 18
# baseline (speedup 1.0000x reference)
"""DGCNN-style graph conv kernel for Trainium2 (8 NeuronCores, data-parallel over batch).

Reference computation (per sample):
  idx = knn(xyz, 20)                        # top-20 by -||xi-xj||^2, per point
  geo = relu(BN1(w1 @ [nb_xyz - xyz; xyz]))
  fea = relu(BN2(w2 @ [nb_feat - feat; feat]))
  out = max_k concat([geo, fea])            # (128, N)

Algebraic collapse (relu/max commute, BN scale > 0):
  out[c, n] = relu( max_k G[c, idx[n, k]] + H[c, n] + hb[c] )
  G = s * (Wa @ X)          (neighbor part, gathered)
  H = s * ((Wb - Wa) @ X)   (center part)
  hb = s * b + shift        (folded BN bias)

Device pipeline per core (1 sample):
  1. G^T (2048, 128) fp16 -> DRAM scratch (matmuls with n on partitions)
  2. D-chunk (128 rows x 2048) = -(dist^2) via one K=5 augmented fp32 matmul
  3. top-20 per row: 3 rounds of (max8, max_index8, match_replace8)
  4. indices -> DRAM in a 16-wrapped layout, reloaded replicated
  5. dma_gather(transpose=True) pulls 256B G^T rows -> ag[c, slot] fp16 on the
     16 DMA engines (~15us/chunk vs ~270us for the gpsimd ap_gather)
  6. tensor_reduce max over k, + H + hb, relu
"""
import numpy as np

B, N, C, K = 8, 2048, 128, 20
H2 = C // 2          # 64
EPS = 1e-5
NEG = -3.0e38
NCHUNK = N // 128    # 16 topk chunks
NGATH = 4            # gather chunks
PTS_G = N // NGATH   # 512 points per gather chunk
NI = PTS_G * K       # 10240 indices per gather chunk

_compiled = None


def _build():
    import concourse.bass as bass
    import concourse.bacc as bacc
    import concourse.mybir as mybir
    import concourse.tile as tile
    from concourse import library_config

    f32 = mybir.dt.float32
    f16 = mybir.dt.float16
    u16 = mybir.dt.uint16

    nc = bacc.Bacc("TRN2")
    xyz_in = nc.declare_dram_parameter("xyz", [3, N], f32, isOutput=False)
    feat_in = nc.declare_dram_parameter("feat", [C, N], f32, isOutput=False)
    wg_xyz_in = nc.declare_dram_parameter("wg_xyz", [3, H2], f32, isOutput=False)
    wg_feat_in = nc.declare_dram_parameter("wg_feat", [C, H2], f32, isOutput=False)
    wh_xyz_in = nc.declare_dram_parameter("wh_xyz", [3, H2], f32, isOutput=False)
    wh_feat_in = nc.declare_dram_parameter("wh_feat", [C, H2], f32, isOutput=False)
    hb_in = nc.declare_dram_parameter("hb", [C, 1], f32, isOutput=False)
    out_dram = nc.declare_dram_parameter("out", [C, N], f32, isOutput=True)

    # wrapped index scratch: row p16 (16 rows), col (b*640 + (c%4)*160 + ph*20 + q)
    idxw_dram = nc.dram_tensor("idxw_scratch", [16, N * K // 16], u16)

    with tile.TileContext(nc) as tc:
        with (
            tc.tile_pool(name="const", bufs=1) as cpool,
            tc.tile_pool(name="work", bufs=2) as wpool,
            tc.tile_pool(name="ag", bufs=2) as agpool,
            tc.tile_pool(name="psum", bufs=2, space="PSUM") as ppool,
        ):
            nc.gpsimd.load_library(library_config.mlp)

            xyz_t = cpool.tile([3, N], f32)
            feat_t = cpool.tile([C, N], f32)
            wgx_t = cpool.tile([3, H2], f32)
            wgf_t = cpool.tile([C, H2], f32)
            whx_t = cpool.tile([3, H2], f32)
            whf_t = cpool.tile([C, H2], f32)
            hb_t = cpool.tile([C, 1], f32)
            nc.sync.dma_start(xyz_t[:], xyz_in[:])
            nc.sync.dma_start(feat_t[:], feat_in[:])
            nc.sync.dma_start(wgx_t[:], wg_xyz_in[:])
            nc.sync.dma_start(wgf_t[:], wg_feat_in[:])
            nc.sync.dma_start(whx_t[:], wh_xyz_in[:])
            nc.sync.dma_start(whf_t[:], wh_feat_in[:])
            nc.sync.dma_start(hb_t[:], hb_in[:])

            # ---- xx[n] = sum_d xyz[d,n]^2 ----
            sq_t = cpool.tile([3, N], f32)
            nc.vector.tensor_tensor(
                out=sq_t[:], in0=xyz_t[:], in1=xyz_t[:], op=mybir.AluOpType.mult
            )
            ones3_t = cpool.tile([3, 1], f32)
            nc.vector.memset(ones3_t[:], 1.0)
            xx_ps = ppool.tile([1, N], f32, space="PSUM", tag="d")
            for j in range(4):
                nc.tensor.matmul(
                    out=xx_ps[:, 512 * j:512 * (j + 1)],
                    lhsT=ones3_t[:],
                    rhs=sq_t[:, 512 * j:512 * (j + 1)],
                    start=True, stop=True,
                )
            xx_t = cpool.tile([1, N], f32)
            nc.scalar.copy(xx_t[:], xx_ps[:])

            # ---- lhs5 = [xyz; xx; 1], rhs5 = [2 xyz; -1; -xx] ----
            # compute-engine ops must start at quadrant-aligned partitions, so
            # rows 3/4 are placed with SBUF->SBUF DMAs instead.
            lhs5 = cpool.tile([5, N], f32)
            rhs5 = cpool.tile([5, N], f32)
            ones_row = cpool.tile([1, N], f32)
            neg1_row = cpool.tile([1, N], f32)
            nxx_t = cpool.tile([1, N], f32)
            nc.vector.memset(ones_row[:], 1.0)
            nc.vector.memset(neg1_row[:], -1.0)
            nc.vector.tensor_scalar_mul(nxx_t[:], xx_t[:], -1.0)
            nc.vector.tensor_copy(lhs5[0:3, :], xyz_t[:])
            nc.vector.tensor_scalar_mul(rhs5[0:3, :], xyz_t[:], 2.0)
            nc.sync.dma_start(lhs5[3:4, :], xx_t[:])
            nc.sync.dma_start(lhs5[4:5, :], ones_row[:])
            nc.sync.dma_start(rhs5[3:4, :], neg1_row[:])
            nc.sync.dma_start(rhs5[4:5, :], nxx_t[:])

            # ---- G (C, N) with stationary weights, then xbar-transpose to
            # gt_sb[p, r*128+c] = G[c, r*128+p] (SBUF-resident G^T rows) ----
            g_ps = ppool.tile([C, N], f32, space="PSUM", tag="d")
            for j in range(4):
                fs = slice(512 * j, 512 * (j + 1))
                nc.tensor.matmul(out=g_ps[0:H2, fs], lhsT=wgx_t[:], rhs=xyz_t[:, fs],
                                 start=True, stop=True)
            for j in range(4):
                fs = slice(512 * j, 512 * (j + 1))
                nc.tensor.matmul(out=g_ps[H2:C, fs], lhsT=wgf_t[:], rhs=feat_t[:, fs],
                                 start=True, stop=True)
            h_t = cpool.tile([C, N], f32)
            g_f16 = cpool.tile([C, N], f16)
            gt_sb = cpool.tile([128, N], f16)
            # sliced copy + transposes split across both HWDGE queues so the
            # first dma_gather (which needs all of gt_sb) starts sooner
            for j in range(4):
                fs = slice(512 * j, 512 * (j + 1))
                nc.scalar.copy(g_f16[:, fs], g_ps[:, fs])
                for rr in range(4):
                    ns = slice(512 * j + 128 * rr, 512 * j + 128 * (rr + 1))
                    eng = nc.sync if rr % 2 == 0 else nc.scalar
                    eng.dma_start_transpose(gt_sb[:, ns], g_f16[:, ns])

            # wrapped idx write view: (16, N*K/16) -> [b, c4, ph, p16, q]
            idxw_w = idxw_dram[:].rearrange(
                "p (b c4 ph q) -> b c4 ph p q", b=NGATH, c4=4, ph=8, q=K
            )

            # ---- per-chunk: D matmul + top-20 ----
            def emit_chunk(c):
                d_ps = ppool.tile([128, N], f32, space="PSUM", tag="d")
                for j in range(4):
                    fs = slice(512 * j, 512 * (j + 1))
                    nc.tensor.matmul(
                        out=d_ps[:, fs],
                        lhsT=lhs5[:, 128 * c:128 * (c + 1)],
                        rhs=rhs5[:, fs],
                        start=True, stop=True,
                    )
                d_sb = wpool.tile([128, N], f32, tag="dsb")
                nc.scalar.copy(d_sb[:], d_ps[:])
                vals = wpool.tile([128, 24], f32, tag="vals", bufs=4)
                idxs = wpool.tile([128, 24], u16, tag="idxs", bufs=4)
                for r in range(3):
                    v8 = vals[:, 8 * r:8 * (r + 1)]
                    i8 = idxs[:, 8 * r:8 * (r + 1)]
                    nc.vector.max(out=v8, in_=d_sb[:])
                    nc.vector.max_index(out=i8, in_max=v8, in_values=d_sb[:])
                    if r < 2:
                        nc.vector.match_replace(
                            out=d_sb[:], in_to_replace=v8, in_values=d_sb[:],
                            imm_value=NEG,
                        )
                # write top-20 indices to wrapped DRAM layout
                nc.sync.dma_start(idxw_w[c // 4, c % 4], idxs[:, 0:K])

            # ---- per-chunk gather (DMA engines), reduces lag 2 rounds ----
            NIC = 128 * K          # 2560 indices per topk chunk
            ag_tiles = {}

            def emit_gather_issue(c):
                # ucode desc-gen runs on Q7 cores 0-1 only; they read the
                # wrapped index list from partitions 0-31 -> 2 replicas.
                idxw_t = agpool.tile([32, NIC // 16], u16, tag="idxw", bufs=6)
                for g in range(2):
                    nc.scalar.dma_start(
                        idxw_t[16 * g:16 * (g + 1), :],
                        idxw_dram[:, (NIC // 16) * c:(NIC // 16) * (c + 1)],
                    )
                ag = agpool.tile([128, NIC], f16, tag="ag", bufs=6)
                ag_tiles[c] = ag
                nc.gpsimd.dma_gather(
                    out_ap=ag[:].rearrange("c (o i) -> c o i", o=1),
                    in_ap=gt_sb[:],
                    idxs_ap=idxw_t[:].bitcast(mybir.dt.int16),
                    num_idxs=NIC,
                    num_idxs_reg=NIC,
                    elem_size=C,
                    transpose=True,
                    single_packet=False,
                    sbuf_tokens_per_rank=128,
                    sbuf_free_dim_per_rank=256,
                )

            def emit_reduce(c):
                ag = ag_tiles.pop(c)
                # slot i = m*320 + q*16 + p16 ; point jj = m*16 + p16 ; reduce over q
                ag4 = ag[:].rearrange("c (m q p) -> c m p q", m=8, q=K, p=16)
                m_t = agpool.tile([128, 128], f32, tag="m")
                nc.vector.tensor_reduce(
                    out=m_t[:], in_=ag4, op=mybir.AluOpType.max,
                    axis=mybir.AxisListType.X,
                )
                ps = slice(128 * c, 128 * (c + 1))
                t_t = agpool.tile([128, 128], f32, tag="t")
                nc.vector.tensor_add(t_t[:], m_t[:], h_t[:, ps])
                o_t = agpool.tile([128, 128], f32, tag="o")
                nc.scalar.activation(
                    o_t[:], t_t[:], mybir.ActivationFunctionType.Relu)
                nc.scalar.dma_start(out_dram[:, ps], o_t[:])

            # ---- H (128, N): emitted after chunk 0 so chunk 0's topk can
            # start as early as possible (PSUM tag-d slot rotation) ----
            def emit_h():
                h_ps = ppool.tile([C, N], f32, space="PSUM", tag="d")
                for j in range(4):
                    fs = slice(512 * j, 512 * (j + 1))
                    nc.tensor.matmul(out=h_ps[0:H2, fs], lhsT=whx_t[:],
                                     rhs=xyz_t[:, fs], start=True, stop=True)
                for j in range(4):
                    fs = slice(512 * j, 512 * (j + 1))
                    nc.tensor.matmul(out=h_ps[H2:C, fs], lhsT=whf_t[:],
                                     rhs=feat_t[:, fs], start=True, stop=True)
                # h_t = H + hb (bias folded during the PSUM->SBUF copy on ACT)
                nc.scalar.activation(
                    h_t[:], h_ps[:], mybir.ActivationFunctionType.Identity,
                    bias=hb_t[:],
                )

            # pipeline: gathers lag the topk by 2 chunks (indices are already
            # in DRAM+SBUF when the gather fires), reduces lag 3 more.
            GLAG, RLAG = 2, 5
            for c in range(NCHUNK):
                if c >= GLAG:
                    emit_gather_issue(c - GLAG)
                if c >= RLAG:
                    emit_reduce(c - RLAG)
                emit_chunk(c)
                if c == 0:
                    emit_h()
            for c in range(NCHUNK - GLAG, NCHUNK):
                emit_gather_issue(c)
            for c in range(NCHUNK - RLAG, NCHUNK):
                emit_reduce(c)

    nc.compile()
    return nc


def _fold_params(w1, b1, g1, be1, m1, v1, w2, b2, g2, be2, m2, v2):
    s1 = g1 / np.sqrt(v1 + EPS)
    sh1 = be1 - m1 * s1
    s2 = g2 / np.sqrt(v2 + EPS)
    sh2 = be2 - m2 * s2
    wg_xyz = (s1[None, :] * w1[:, 0:3].T).astype(np.float32)        # (3, 64)
    wh_xyz = (s1[None, :] * (w1[:, 3:6] - w1[:, 0:3]).T).astype(np.float32)
    wg_feat = (s2[None, :] * w2[:, 0:C].T).astype(np.float32)       # (128, 64)
    wh_feat = (s2[None, :] * (w2[:, C:2 * C] - w2[:, 0:C]).T).astype(np.float32)
    hb = np.concatenate([s1 * b1 + sh1, s2 * b2 + sh2]).astype(np.float32)[:, None]
    return wg_xyz, wg_feat, wh_xyz, wh_feat, hb


def kernel(xyz, features, w1, b1, g1, be1, m1, v1, w2, b2, g2, be2, m2, v2, k):
    global _compiled
    assert int(k) == K
    from concourse.bass_utils import run_bass_kernel_spmd

    if _compiled is None:
        _compiled = _build()
    nc = _compiled

    wg_xyz, wg_feat, wh_xyz, wh_feat, hb = _fold_params(
        np.asarray(w1), np.asarray(b1), np.asarray(g1), np.asarray(be1),
        np.asarray(m1), np.asarray(v1), np.asarray(w2), np.asarray(b2),
        np.asarray(g2), np.asarray(be2), np.asarray(m2), np.asarray(v2),
    )
    xyz = np.ascontiguousarray(np.asarray(xyz, dtype=np.float32))
    features = np.ascontiguousarray(np.asarray(features, dtype=np.float32))

    in_maps = []
    for bb in range(B):
        in_maps.append({
            "xyz": xyz[bb],
            "feat": features[bb],
            "wg_xyz": wg_xyz, "wg_feat": wg_feat,
            "wh_xyz": wh_xyz, "wh_feat": wh_feat,
            "hb": hb,
        })
    res = run_bass_kernel_spmd(nc, in_maps, list(range(B)))
    out = np.stack([res.results[bb]["out"] for bb in range(B)], axis=0)
    return out.astype(np.float32)
